# revision 32
# baseline (speedup 1.0000x reference)
"""Trainium2 Bass kernel for nn_DensePoseV1ConvXGNSparseGNHead.

Reference computation (per layer l in 0..7):
    y = x @ W[l] + b[l]
    per-instance GroupNorm over (tokens-of-instance, group-channels)
    x = xn * gamma[l] + beta[l]
    per-instance ECA: m = mean_tokens(x); conv1d(k=3) over channels; gate=sigmoid
    x = relu(x * gate[ids])

Strategy:
  * Host sorts points by instance id. 64 instances -> 8 cores x 8 slots.
    Each slot is padded to S columns (S >= max instance size). Zero
    cross-core communication is needed.
  * On-chip layout: x is [channel, point] = 2 blocks of 128 partitions,
    points along the free dim. Matmul per layer: lhsT = W chunk
    [c_in(128), c_out(128)], rhs = x chunk [c_in(128), cols], PSUM out
    [c_out(128), cols], accumulated over the 2 c_in chunks.
  * Per layer (fp16 data), work is spread across ALL engines:
      - PSUM->SBUF drain (cast fp16): split between ScalarE (ACT copy)
        and GpSimd (Pool copy).  NO accumulation on the drain.
      - s1 (per-instance channel sums of y) comes analytically from the
        PREVIOUS layer's normalize accum_out:  s1 = sigma1 @ W (+ tiny
        corrections), a [P,8]-wide PE matmul.  Layer 0 uses host sums.
      - s2 (sums of squares): scalar_tensor_tensor on VectorE (no DVE
        perf mode, 1x) / ACT Square+accum / Pool stt, split for balance.
      - normalize+affine+gate+relu fused into ONE VectorE tensor_scalar
        (4x DVE perf mode):  relu(y*A + B) = max(y*A, -B) + B, with the
        +B shift carried analytically into the next layer.  Its
        accum_out produces sigma1 for the next layer's s1 for free.
      - Padded columns hold a per-slot constant vector also tracked
        analytically; small-domain stats run on block-combined [128,16]
        tensors (one op for both channel blocks).
  * Transcendentals: sigmoid on ACT (one [128,16] op per layer); rsqrt
    via int-magic seed + 3 Newton steps on VectorE.

The host applies the final +B shift and un-sorts/un-pads the output.
"""

import sys

sys.path.insert(0, "/opt/trn_rl_repo")

import numpy as np

import concourse.tile as tile
from concourse import bacc, mybir

# ---------------------------------------------------------------- constants
N = 120000
C = 256
L = 8
G = 32          # groups
GS = C // G     # 8 channels per group
K = 3
NUM_INS = 64
EPS = 1e-5
NCORES = 8
IPC = NUM_INS // NCORES      # instances (slots) per core = 8
NBLK = 2                     # channel blocks of 128
P = 128
SC = NBLK * IPC              # small-tensor width (block-combined) = 16

HALF = mybir.dt.float16
NP_HALF = np.float16
F32 = mybir.dt.float32

AF = mybir.ActivationFunctionType
OP = mybir.AluOpType

_PROGRAM_CACHE = {}
LAST_RESULTS = None   # test.py introspection

# engine split knobs (tuned via TimelineSim).  GpSimd cannot touch PSUM,
# so drains are ACT (or DVE); Pool only takes SBUF-side sumsq work.
DRAIN_PATTERN = "aaaaaaaaaaaaaaaa"   # per drain unit: a=ACT d=DVE
SUMSQ_PATTERN = "dadd"               # per (slot,block): d=DVE a=ACT


# ================================================================ device IR
def build_program(S: int, n_layers: int = L, reps: int = 1):
    """Build + compile the per-core Bass program for slot size S."""
    NC_COLS = IPC * S
    PFD = 2048                   # psum tile free size (4 banks)
    NH = -(-S // PFD)            # psum tiles per (slot, block)
    CHUNKS = [(h * PFD, min(PFD, S - h * PFD)) for h in range(NH)]

    nc = bacc.Bacc(
        "TRN2", target_bir_lowering=False, debug=False,
        enable_asserts=False, num_devices=NCORES,
    )

    # ---- DRAM I/O
    x0_d = nc.dram_tensor("x0", [NBLK, P, NC_COLS], HALF, kind="ExternalInput")
    w_d = nc.dram_tensor("wt", [P, L * NBLK * C], HALF, kind="ExternalInput")
    band_d = nc.dram_tensor("band", [P, L * NBLK * C], F32, kind="ExternalInput")
    gam_d = nc.dram_tensor("gamT", [P, L * SC], F32, kind="ExternalInput")
    bet_d = nc.dram_tensor("betT", [P, L * SC], F32, kind="ExternalInput")
    bia_d = nc.dram_tensor("biaT", [P, L * SC], F32, kind="ExternalInput")
    ggat_d = nc.dram_tensor("ggat", [P, G // NBLK], F32, kind="ExternalInput")
    gsca_d = nc.dram_tensor("gsca", [G // NBLK, P], F32, kind="ExternalInput")
    nvec_d = nc.dram_tensor("nvec", [P, SC], F32, kind="ExternalInput")
    rn_d = nc.dram_tensor("rn", [P, SC], F32, kind="ExternalInput")
    rnc_d = nc.dram_tensor("rnc", [P, SC], F32, kind="ExternalInput")
    pc_d = nc.dram_tensor("pc", [P, SC], F32, kind="ExternalInput")
    xout_d = nc.dram_tensor("xout", [NBLK, P, NC_COLS], HALF, kind="ExternalOutput")
    bout_d = nc.dram_tensor("bout", [P, SC], F32, kind="ExternalOutput")
    dbg_d = nc.dram_tensor("dbg", [P, 208], F32, kind="ExternalOutput") if n_layers <= 2 else None

    GPB = G // NBLK  # groups per block = 16

    with tile.TileContext(nc) as tc:
        with (
            tc.tile_pool(name="persist", bufs=1) as persist,
            tc.tile_pool(name="xy", bufs=1) as xy,
            tc.tile_pool(name="ysq", bufs=4) as ysqp,
            tc.tile_pool(name="stage", bufs=2) as stage,
            tc.tile_pool(name="sm", bufs=4) as sm,
            tc.tile_pool(name="carry", bufs=2) as carry,
            tc.tile_pool(name="ps", bufs=2, space="PSUM") as psb,
        ):
            # ---------- persistent constants
            w_sb = persist.tile([P, L * NBLK * C], HALF, tag="w")
            nc.sync.dma_start(w_sb[:], w_d.ap())
            band_sb = persist.tile([P, L * NBLK * C], F32, tag="band")
            nc.sync.dma_start(band_sb[:], band_d.ap())
            gam_sb = persist.tile([P, L * SC], F32, tag="gam")
            nc.sync.dma_start(gam_sb[:], gam_d.ap())
            bet_sb = persist.tile([P, L * SC], F32, tag="bet")
            nc.sync.dma_start(bet_sb[:], bet_d.ap())
            bia_sb = persist.tile([P, L * SC], F32, tag="bia")
            nc.sync.dma_start(bia_sb[:], bia_d.ap())
            ggat_sb = persist.tile([P, GPB], F32, tag="ggat")
            nc.sync.dma_start(ggat_sb[:], ggat_d.ap())
            gsca_sb = persist.tile([GPB, P], F32, tag="gsca")
            nc.sync.dma_start(gsca_sb[:], gsca_d.ap())
            nvec_sb = persist.tile([P, SC], F32, tag="nvec")
            nc.sync.dma_start(nvec_sb[:], nvec_d.ap())
            rn_sb = persist.tile([P, SC], F32, tag="rn")
            nc.sync.dma_start(rn_sb[:], rn_d.ap())
            rnc_sb = persist.tile([P, SC], F32, tag="rnc")
            nc.sync.dma_start(rnc_sb[:], rnc_d.ap())
            pc_sb = persist.tile([P, SC], F32, tag="pc")
            nc.sync.dma_start(pc_sb[:], pc_d.ap())

            def wchunk(l, k, b):
                # lhsT [c_in(128) of chunk k, c_out 128b:128b+128]
                return w_sb[:, (l * NBLK + k) * C + b * P:(l * NBLK + k) * C + (b + 1) * P]

            def bandchunk(l, k, b):
                return band_sb[:, (l * NBLK + k) * C + b * P:(l * NBLK + k) * C + (b + 1) * P]

            def lsl(l):  # per-layer [P, SC] slice of gam/bet/bia
                return slice(l * SC, (l + 1) * SC)

            # ---------- big data
            x_sb = [xy.tile([P, NC_COLS], HALF, tag=f"x{b}", name=f"xsb{b}") for b in range(NBLK)]
            y_sb = [xy.tile([P, NC_COLS], HALF, tag=f"y{b}", name=f"ysb{b}") for b in range(NBLK)]

            def one_run(rep):
                for b in range(NBLK):
                    for s in range(IPC):
                        nc.sync.dma_start(
                            x_sb[b][:, s * S:(s + 1) * S],
                            x0_d.ap()[b, :, s * S:(s + 1) * S],
                        )

                # ---------- cross-layer carried state (block-combined [P, SC])
                v_prev = carry.tile([P, SC], HALF, tag="v", name=f"vprev{rep}")
                bq_prev = carry.tile([P, SC], HALF, tag="bq", name=f"bqprev{rep}")
                nc.vector.memset(v_prev[:], 0.0)
                nc.vector.memset(bq_prev[:], 0.0)

                for l in range(n_layers):
                    last = l == n_layers - 1
                    # ---- tiny PE matmuls (into a big-pool PSUM tile, cols 0:32):
                    #  cols 0:16 d = Bq_prev@W | 16:32 vp = v_prev@W
                    # NOTE: each PSUM accumulation group's matmuls must be
                    # emitted consecutively — interleaving start/stop groups
                    # in one bank silently drops the start contribution.
                    tm_ps = psb.tile([P, PFD], F32, tag="big", name=f"tm{rep}_{l}")
                    for b in range(NBLK):
                        for k in range(NBLK):
                            nc.tensor.matmul(tm_ps[:, b * 8:b * 8 + 8], wchunk(l, k, b),
                                             bq_prev[:, k * 8:k * 8 + 8],
                                             start=(k == 0), stop=(k == 1))
                        for k in range(NBLK):
                            nc.tensor.matmul(tm_ps[:, 16 + b * 8:16 + b * 8 + 8], wchunk(l, k, b),
                                             v_prev[:, k * 8:k * 8 + 8],
                                             start=(k == 0), stop=(k == 1))
                    tm_sb = sm.tile([P, 32], F32, tag="tm", name=f"tmsb{rep}_{l}")
                    nc.vector.tensor_copy(tm_sb[:], tm_ps[:, 0:32])
                    d16 = tm_sb[:, 0:16]
                    vpf = tm_sb[:, 16:32]
                    vpb = sm.tile([P, SC], HALF, tag="vpb", name=f"vpb{rep}_{l}")
                    nc.vector.tensor_copy(vpb[:], vpf)

                    # ---- main matmuls + drains w/ s1 accum (ACT/DVE) + sumsq
                    s1h = sm.tile([P, SC * NH], F32, tag="s1h", name=f"s1h{rep}_{l}")
                    s2t = sm.tile([P, SC], F32, tag="s2t", name=f"s2t{rep}_{l}")
                    du = 0   # drain unit counter
                    for s in range(IPC):
                        for b in range(NBLK):
                            col = b * 8 + s
                            for h, (hoff, clen) in enumerate(CHUNKS):
                                pt = psb.tile([P, PFD], F32, tag="big")
                                c0 = s * S + hoff
                                for q0 in range(0, clen, 512):
                                    qn = min(512, clen - q0)
                                    for k in range(NBLK):
                                        nc.tensor.matmul(
                                            pt[:, q0:q0 + qn],
                                            wchunk(l, k, b),
                                            x_sb[k][:, c0 + q0:c0 + q0 + qn],
                                            start=(k == 0), stop=(k == 1),
                                        )
                                acc_ap = s1h[:, col * NH + h:col * NH + h + 1]
                                if DRAIN_PATTERN[du % len(DRAIN_PATTERN)] == "d":
                                    # accum form: out = in*1 ; acc = sum(out)
                                    nc.vector.tensor_scalar(
                                        out=y_sb[b][:, c0:c0 + clen],
                                        in0=pt[:, :clen], scalar1=1.0, scalar2=None,
                                        op0=OP.mult, op1=OP.add, accum_out=acc_ap)
                                else:
                                    nc.scalar.activation(
                                        y_sb[b][:, c0:c0 + clen], pt[:, :clen],
                                        AF.Copy, accum_out=acc_ap)
                                du += 1
                            # sum of squares for the whole (slot, block)
                            col = b * 8 + s
                            eng = SUMSQ_PATTERN[(s * NBLK + b) % len(SUMSQ_PATTERN)]
                            yv = y_sb[b][:, s * S:(s + 1) * S]
                            if eng == "a":
                                ysq = ysqp.tile([P, S], HALF, tag="ysq",
                                                name=f"ysq{rep}_{l}_{s}_{b}")
                                nc.scalar.activation(ysq[:], yv, AF.Square,
                                                     accum_out=s2t[:, col:col + 1])
                            elif eng == "p":
                                ysq = ysqp.tile([P, S], HALF, tag="ysq",
                                                name=f"ysq{rep}_{l}_{s}_{b}")
                                nc.gpsimd.scalar_tensor_tensor(
                                    out=ysq[:], in0=yv, scalar=1.0, in1=yv,
                                    op0=OP.mult, op1=OP.mult,
                                    accum_out=s2t[:, col:col + 1])
                            else:
                                ysq = ysqp.tile([P, S], HALF, tag="ysq",
                                                name=f"ysq{rep}_{l}_{s}_{b}")
                                nc.vector.scalar_tensor_tensor(
                                    out=ysq[:], in0=yv, scalar=1.0, in1=yv,
                                    op0=OP.mult, op1=OP.mult,
                                    accum_out=s2t[:, col:col + 1])

                    # ---- block-combined small-domain statistics [P, SC]
                    t0 = sm.tile([P, SC], F32, tag="t0", name=f"t0{rep}_{l}")
                    t1 = sm.tile([P, SC], F32, tag="t1", name=f"t1{rep}_{l}")
                    s1 = sm.tile([P, SC], F32, tag="s1", name=f"s1{rep}_{l}")
                    s2 = sm.tile([P, SC], F32, tag="s2", name=f"s2{rep}_{l}")
                    ee = sm.tile([P, SC], F32, tag="ee", name=f"ee{rep}_{l}")
                    # E = D + bias ; s1 = s1raw - pc*vp + n*E
                    if NH == 1:
                        s1raw = s1h[:]
                    else:
                        s1raw = s1[:]  # accumulate strided halves into s1
                        nc.vector.tensor_tensor(
                            out=s1raw, in0=s1h[:, 0::NH], in1=s1h[:, 1::NH], op=OP.add)
                        for h in range(2, NH):
                            nc.vector.tensor_tensor(
                                out=s1raw, in0=s1raw, in1=s1h[:, h::NH], op=OP.add)
                    nc.vector.tensor_tensor(out=ee[:], in0=d16, in1=bia_sb[:, lsl(l)], op=OP.add)
                    nc.vector.tensor_tensor(out=t0[:], in0=pc_sb[:], in1=vpf, op=OP.mult)
                    nc.vector.tensor_tensor(out=s1[:], in0=s1raw, in1=t0[:], op=OP.subtract)
                    nc.vector.tensor_tensor(out=t0[:], in0=nvec_sb[:], in1=ee[:], op=OP.mult)
                    nc.vector.tensor_tensor(out=s1[:], in0=s1[:], in1=t0[:], op=OP.add)
                    # s2 = s2t - pc*vpb^2 + 2*E*(s1 - n*E) + n*E^2
                    #    = s2t - pc*vpb^2 + 2*E*s1 - n*E^2
                    nc.vector.tensor_tensor(out=t0[:], in0=vpb[:], in1=vpb[:], op=OP.mult)
                    nc.vector.tensor_tensor(out=t0[:], in0=t0[:], in1=pc_sb[:], op=OP.mult)
                    nc.vector.tensor_tensor(out=s2[:], in0=s2t[:], in1=t0[:], op=OP.subtract)
                    nc.vector.tensor_tensor(out=t0[:], in0=ee[:], in1=s1[:], op=OP.mult)
                    nc.vector.tensor_scalar(out=t0[:], in0=t0[:], scalar1=2.0,
                                            scalar2=None, op0=OP.mult)
                    nc.vector.tensor_tensor(out=s2[:], in0=s2[:], in1=t0[:], op=OP.add)
                    nc.vector.tensor_tensor(out=t1[:], in0=ee[:], in1=ee[:], op=OP.mult)
                    nc.vector.tensor_tensor(out=t1[:], in0=t1[:], in1=nvec_sb[:], op=OP.mult)
                    nc.vector.tensor_tensor(out=s2[:], in0=s2[:], in1=t1[:], op=OP.subtract)

                    # group aggregation (PE one-hot, 1/GS folded into ggat)
                    gg_ps = psb.tile([P, PFD], F32, tag="big", name=f"gg{rep}_{l}")
                    nc.tensor.matmul(gg_ps[:GPB, 0:16], ggat_sb[:], s1[:])
                    nc.tensor.matmul(gg_ps[:GPB, 16:32], ggat_sb[:], s2[:])
                    gm = sm.tile([P, SC], F32, tag="gm", name=f"gm{rep}_{l}")
                    gv = sm.tile([P, SC], F32, tag="gv", name=f"gv{rep}_{l}")
                    # mu_g = Sg1*rn ; E2_g = Sg2*rn ; var = E2 - mu^2 (+eps)
                    nc.vector.tensor_tensor(out=gm[:GPB, :], in0=gg_ps[:GPB, 0:16],
                                            in1=rn_sb[:GPB, :], op=OP.mult)
                    nc.vector.tensor_tensor(out=gv[:GPB, :], in0=gg_ps[:GPB, 16:32],
                                            in1=rn_sb[:GPB, :], op=OP.mult)
                    nc.vector.tensor_tensor(out=t0[:GPB, :], in0=gm[:GPB, :],
                                            in1=gm[:GPB, :], op=OP.mult)
                    nc.vector.tensor_tensor(out=gv[:GPB, :], in0=gv[:GPB, :],
                                            in1=t0[:GPB, :], op=OP.subtract)
                    nc.vector.tensor_scalar(out=gv[:GPB, :], in0=gv[:GPB, :],
                                            scalar1=EPS, scalar2=None, op0=OP.add)
                    # inv_g = rsqrt(var + eps): int-magic seed + 3 Newton
                    rs = sm.tile([P, SC], F32, tag="rs", name=f"rs{rep}_{l}")
                    rt = sm.tile([P, SC], F32, tag="rt", name=f"rt{rep}_{l}")
                    rsu = rs[:GPB, :].bitcast(mybir.dt.uint32)
                    nc.vector.tensor_scalar(
                        out=rsu, in0=gv[:GPB, :].bitcast(mybir.dt.uint32),
                        scalar1=1, scalar2=None, op0=OP.logical_shift_right)
                    nc.vector.tensor_scalar(out=rsu, in0=rsu, scalar1=0x7FFFFFFF,
                                            scalar2=None, op0=OP.bitwise_xor)
                    nc.vector.tensor_scalar(out=rsu, in0=rsu,
                                            scalar1=0x7FFFFFFF - 0x5F3759DF,
                                            scalar2=None, op0=OP.subtract)
                    for _ in range(3):
                        nc.vector.tensor_tensor(out=rt[:GPB, :], in0=rs[:GPB, :],
                                                in1=rs[:GPB, :], op=OP.mult)
                        nc.vector.tensor_tensor(out=rt[:GPB, :], in0=rt[:GPB, :],
                                                in1=gv[:GPB, :], op=OP.mult)
                        nc.vector.tensor_scalar(out=rt[:GPB, :], in0=rt[:GPB, :],
                                                scalar1=-0.5, scalar2=1.5,
                                                op0=OP.mult, op1=OP.add)
                        nc.vector.tensor_tensor(out=rs[:GPB, :], in0=rs[:GPB, :],
                                                in1=rt[:GPB, :], op=OP.mult)
                    # broadcast groups -> channels (one PE matmul pair)
                    bc_ps = psb.tile([P, PFD], F32, tag="big", name=f"bc{rep}_{l}")
                    nc.tensor.matmul(bc_ps[:, 0:16], gsca_sb[:], rs[:GPB, :])
                    nc.tensor.matmul(bc_ps[:, 16:32], gsca_sb[:], gm[:GPB, :])
                    ivmu = sm.tile([P, 32], F32, tag="ivmu", name=f"ivmu{rep}_{l}")
                    nc.vector.tensor_copy(ivmu[:], bc_ps[:, 0:32])
                    iv = ivmu[:, 0:16]
                    mu = ivmu[:, 16:32]

                    # ECA mean (affine-transformed): ((s1*rn) - mu) * iv * gam + bet
                    maff = sm.tile([P, SC], F32, tag="maff", name=f"maff{rep}_{l}")
                    nc.vector.tensor_tensor(out=t0[:], in0=s1[:], in1=rnc_sb[:], op=OP.mult)
                    nc.vector.tensor_tensor(out=t0[:], in0=t0[:], in1=mu, op=OP.subtract)
                    nc.vector.tensor_tensor(out=t0[:], in0=t0[:], in1=iv, op=OP.mult)
                    nc.vector.tensor_tensor(out=t0[:], in0=t0[:], in1=gam_sb[:, lsl(l)],
                                            op=OP.mult)
                    nc.vector.tensor_tensor(out=maff[:], in0=t0[:], in1=bet_sb[:, lsl(l)],
                                            op=OP.add)

                    # ECA conv across channels (PE banded matmul) + sigmoid gate
                    cv_ps = psb.tile([P, PFD], F32, tag="big", name=f"cv{rep}_{l}")
                    for b in range(NBLK):
                        for k in range(NBLK):
                            nc.tensor.matmul(cv_ps[:, b * 8:b * 8 + 8],
                                             bandchunk(l, k, b), maff[:, k * 8:k * 8 + 8],
                                             start=(k == 0), stop=(k == 1))
                    gate = sm.tile([P, SC], F32, tag="gate", name=f"gate{rep}_{l}")
                    nc.scalar.activation(gate[:], cv_ps[:, 0:16], AF.Sigmoid,
                                         bias=0.0, scale=1.0)

                    # A = iv*gam*gate ; B = ((E-mu)*iv*gam + bet)*gate
                    a16 = sm.tile([P, SC], F32, tag="a16", name=f"a16{rep}_{l}")
                    nb16 = sm.tile([P, SC], F32, tag="nb16", name=f"nb16{rep}_{l}")
                    bq_new = carry.tile([P, SC], HALF, tag="bq", name=f"bqn{rep}_{l}")
                    v_new = carry.tile([P, SC], HALF, tag="v", name=f"vn{rep}_{l}")
                    nc.vector.tensor_tensor(out=t0[:], in0=iv, in1=gate[:], op=OP.mult)
                    nc.vector.tensor_tensor(out=a16[:], in0=t0[:], in1=gam_sb[:, lsl(l)],
                                            op=OP.mult)
                    nc.vector.tensor_tensor(out=t1[:], in0=ee[:], in1=mu, op=OP.subtract)
                    nc.vector.tensor_tensor(out=t1[:], in0=t1[:], in1=iv, op=OP.mult)
                    nc.vector.tensor_tensor(out=t1[:], in0=t1[:], in1=gam_sb[:, lsl(l)],
                                            op=OP.mult)
                    nc.vector.tensor_tensor(out=t1[:], in0=t1[:], in1=bet_sb[:, lsl(l)],
                                            op=OP.add)
                    nc.vector.tensor_tensor(out=t1[:], in0=t1[:], in1=gate[:], op=OP.mult)
                    nc.vector.tensor_copy(bq_new[:], t1[:])          # quantize fp16
                    nc.vector.tensor_scalar(out=nb16[:], in0=bq_new[:],
                                            scalar1=-1.0, scalar2=None, op0=OP.mult)
                    # v_next = max(vpb * A, -Bq)   (matches padded columns)
                    nc.vector.tensor_tensor(out=t0[:], in0=vpb[:], in1=a16[:], op=OP.mult)
                    nc.vector.tensor_tensor(out=v_new[:], in0=t0[:], in1=nb16[:], op=OP.max)
                    if dbg_d is not None:
                        dbg = sm.tile([P, 208], F32, tag="dbg", name=f"dbg{rep}_{l}")
                        nc.vector.tensor_copy(dbg[:, 0:16], s1[:])
                        nc.vector.tensor_copy(dbg[:, 16:32], s2[:])
                        nc.vector.tensor_copy(dbg[:, 32:48], s2t[:])
                        nc.vector.tensor_copy(dbg[:, 48:64], a16[:])
                        nc.vector.tensor_copy(dbg[:, 64:80], nb16[:])
                        nc.vector.tensor_copy(dbg[:, 80:96], bq_prev[:])
                        nc.vector.tensor_copy(dbg[:, 96:112], ivmu[:, 0:16])
                        nc.vector.tensor_copy(dbg[:, 112:128], ivmu[:, 16:32])
                        nc.vector.tensor_copy(dbg[:, 128:144], maff[:])
                        nc.vector.tensor_copy(dbg[:, 144:160], s1h[:, 0:16] if NH == 1 else s1[:])
                        nc.vector.tensor_copy(dbg[:, 160:176], d16)
                        nc.vector.tensor_copy(dbg[:, 176:192], vpf)
                        nc.vector.tensor_copy(dbg[:, 192:208], vpb[:])
                        nc.sync.dma_start(dbg_d.ap(), dbg[:])
                    if last:
                        nc.sync.dma_start(bout_d.ap(), nb16[:])

                    # ---- fused normalize+gate+relu:  x' = max(y'*A, -Bq)
                    for s in range(IPC):
                        for b in range(NBLK):
                            col = b * 8 + s
                            if not last:
                                nc.vector.tensor_scalar(
                                    out=x_sb[b][:, s * S:(s + 1) * S],
                                    in0=y_sb[b][:, s * S:(s + 1) * S],
                                    scalar1=a16[:, col:col + 1],
                                    scalar2=nb16[:, col:col + 1],
                                    op0=OP.mult, op1=OP.max)
                            else:
                                ot = stage.tile([P, S], HALF, tag="out")
                                nc.vector.tensor_scalar(
                                    out=ot[:],
                                    in0=y_sb[b][:, s * S:(s + 1) * S],
                                    scalar1=a16[:, col:col + 1],
                                    scalar2=nb16[:, col:col + 1],
                                    op0=OP.mult, op1=OP.max)
                                nc.sync.dma_start(
                                    xout_d.ap()[b, :, s * S:(s + 1) * S], ot[:])
                    v_prev, bq_prev = v_new, bq_new

            for rep in range(reps):
                one_run(rep)

    nc.compile()
    return nc


# ================================================================ host side
def _prepare(features, W, b, gamma, beta, eca_w, ins_indices):
    counts = np.bincount(ins_indices, minlength=NUM_INS).astype(np.int64)
    order = np.argsort(ins_indices, kind="stable")
    starts = np.zeros(NUM_INS + 1, np.int64)
    np.cumsum(counts, out=starts[1:])

    S = int(max(1088, -(-int(counts.max()) // 64) * 64))
    NC_COLS = IPC * S

    feat_s = np.ascontiguousarray(features[order].T)        # [C, N] sorted
    feat_bf = feat_s.astype(NP_HALF)

    x0 = np.zeros((NCORES, NBLK, P, NC_COLS), NP_HALF)
    nvec = np.zeros((NCORES, P, SC), np.float32)
    rn = np.zeros((NCORES, P, SC), np.float32)
    rnc = np.zeros((NCORES, P, SC), np.float32)
    pc = np.zeros((NCORES, P, SC), np.float32)
    for c in range(NCORES):
        for i in range(IPC):
            g = c * IPC + i
            n = int(counts[g])
            if n:
                x0[c, 0, :, i * S:i * S + n] = feat_bf[:P, starts[g]:starts[g] + n]
                x0[c, 1, :, i * S:i * S + n] = feat_bf[P:, starts[g]:starts[g] + n]
            for bb in range(NBLK):
                nvec[c, :, bb * 8 + i] = float(n)
                rn[c, :, bb * 8 + i] = 1.0 / float(max(n, 1)) / GS
                rnc[c, :, bb * 8 + i] = 1.0 / float(max(n, 1))
                pc[c, :, bb * 8 + i] = float(S - n)

    # [P, (l k c)]: wt[p, (l,k,c)] = W[l][k*128+p, c]
    wt = np.ascontiguousarray(
        np.asarray(W, np.float32).reshape(L, NBLK, P, C)
        .transpose(2, 0, 1, 3).reshape(P, L * NBLK * C)).astype(NP_HALF)

    # conv[c'] = sum_j w_j * m[c'+j-1]  =>  T[cin, cout]=w_j with cout=cin-j+1
    band = np.zeros((L, C, C), np.float32)
    idx = np.arange(C)
    for j in range(K):
        d = K // 2 - j   # cout - cin
        cin = idx[(idx + d >= 0) & (idx + d < C)]
        band[:, cin, cin + d] = np.asarray(eca_w, np.float32)[:, j][:, None]
    band = np.ascontiguousarray(
        band.reshape(L, NBLK, P, C).transpose(2, 0, 1, 3).reshape(P, L * NBLK * C))

    def _perchan16(a):  # [L, C] -> [P, L*SC] replicated per slot
        v = np.asarray(a, np.float32).reshape(L, NBLK, P)   # [l, b, p]
        out = np.zeros((P, L, SC), np.float32)
        for l in range(L):
            for bb in range(NBLK):
                out[:, l, bb * 8:(bb + 1) * 8] = v[l, bb][:, None]
        return np.ascontiguousarray(out.reshape(P, L * SC))
    gamT = _perchan16(gamma)
    betT = _perchan16(beta)
    biaT = _perchan16(b)

    gpb = G // NBLK
    ggat = np.zeros((P, gpb), np.float32)
    ggat[np.arange(P), np.arange(P) // GS] = 1.0   # 1/GS folded into rn
    gsca = np.ascontiguousarray(ggat.T)

    shared = dict(wt=wt, band=band, gamT=gamT, betT=betT, biaT=biaT,
                  ggat=ggat, gsca=gsca)
    in_maps = []
    for c in range(NCORES):
        m = dict(shared)
        m.update(x0=np.ascontiguousarray(x0[c]), nvec=np.ascontiguousarray(nvec[c]),
                 rn=np.ascontiguousarray(rn[c]), rnc=np.ascontiguousarray(rnc[c]),
                 pc=np.ascontiguousarray(pc[c]))
        in_maps.append(m)
    return in_maps, counts, order, starts, S


def _assemble(results, counts, order, starts, S, n_points=N):
    out = np.empty((n_points, C), np.float32)
    for c in range(NCORES):
        xo = results[c]["xout"]          # [2, 128, NC_COLS] fp16 (= x')
        nb = results[c]["bout"]          # [128, 16] fp32 (= -Bq)
        for i in range(IPC):
            g = c * IPC + i
            n = int(counts[g])
            if n == 0:
                continue
            blk = np.empty((NBLK, P, n), np.float32)
            for bb in range(NBLK):
                blk[bb] = (xo[bb, :, i * S:i * S + n].astype(np.float32)
                           - nb[:, bb * 8 + i:bb * 8 + i + 1])
            out[order[starts[g]:starts[g] + n], :] = blk.reshape(C, n).T
    return out


def kernel(features, W, b, gamma, beta, eca_w, ins_indices):
    global LAST_RESULTS
    features = np.asarray(features, np.float32)
    W = np.asarray(W, np.float32)
    b = np.asarray(b, np.float32)
    gamma = np.asarray(gamma, np.float32)
    beta = np.asarray(beta, np.float32)
    eca_w = np.asarray(eca_w, np.float32)
    ins_indices = np.asarray(ins_indices, np.int32)

    in_maps, counts, order, starts, S = _prepare(
        features, W, b, gamma, beta, eca_w, ins_indices)

    if S not in _PROGRAM_CACHE:
        _PROGRAM_CACHE[S] = build_program(S)
    nc = _PROGRAM_CACHE[S]

    from concourse import bass_utils
    res = bass_utils.run_bass_kernel_spmd(
        nc, in_maps, core_ids=list(range(NCORES)), trace=False)
    LAST_RESULTS = res
    return _assemble(res.results, counts, order, starts, S,
                     n_points=features.shape[0])
